# revision 38
# baseline (speedup 1.0000x reference)
"""BitLinear (RMSNorm + int8 act quant + ternary weight quant + GEMM) on 8 TRN2 cores.

Sharding: 2 token-groups x 4 dout-groups. Each core:
  - x shard [4096, 2048] (token-parallel)
  - wT shard [2048, 2048] = weight[og*2048:(og+1)*2048, :].T  (host pre-transposed layout)
  - wsc shard [1024, 2048] = weight[c*1024:(c+1)*1024, :]     (for global mean|w| AllReduce)

Device pipeline per core (v6):
  A: warmup AllReduce issued immediately (absorbs the ~50-70us ncfw arming +
     launch skew); wsc streamed + Abs-accum; real AllReduce ~15us right after.
  B: x-tile stats for the first tiles run through the arming window; the
     per-token scalar chain lives on GpSimd, reciprocals on DVE, sumsq/sqrt
     on ACT.  norm_weight==1 fast path skips the x*gw multiply entirely.
  C: weight quant in 2 ALU ops per [128,512] chunk:
       c  = clip(w, -ws, +ws)                (DVE/Pool split by k parity)
       wq = bf16(c*inv_ws + 192)             (ACT/DVE split, RNE to {191,192,193})
     +192 keeps every value bf16-exact (ulp=1 in [128,256)).  psum then equals
     true_mm + 192*rowsum(x_q); fixed for free in the PSUM->SBUF copy:
       out = psum*alpha + (-192*alpha*rowsum(x_q))
     rowsum(x_q) rides the qt8 pass as a DVE accumulator.
  D: first two tiles' matmuls run oc-major so early oc-chunk deadlines trail
     the quant pipeline; rest per-tile, interleaved with remaining stats.
     Output DMA per 512-col chunk.
The quantized GEMM stays exact: x_q in [-127,127], offset weights in
{191,192,193} are bf16-exact, PSUM accumulates fp32 (partials ~1e6, well
under 2^24; the 192-offset costs ~1e-3 relative rounding worst case, far
inside the 2e-2 gate).
"""

import sys

if "/opt/trn_rl_repo" not in sys.path:
    sys.path.insert(0, "/opt/trn_rl_repo")

import numpy as np

# ---------------------------------------------------------------- config

N_CORES = 8
TG, OG = 2, 4            # token groups x dout groups
B, S, DIN, DOUT = 4, 2048, 2048, 8192
TOKENS = B * S           # 8192
T_SH = TOKENS // TG      # 4096 tokens per core
O_SH = DOUT // OG        # 2048 dout per core
WSC_ROWS = DOUT // N_CORES  # 1024 rows of w per core for the scale pass

P = 128                  # partitions
EPS_NORM = 1e-6
EPS_SCALE = 1e-8
QB = 127.0
C_MAGIC = 12582912.0     # 1.5 * 2^23 : float32 RNE integer-rounding constant
W_OFF = 192.0            # bf16 magic: ints exact (ulp=1) in [128, 256)
N_W = float(DOUT * DIN)  # elements of weight for the global mean

N_EARLY = 4              # x tiles with stats emitted before weight quant
N_GROUP = 2              # leading tiles run oc-major (must be << qT pool
                         # depth: the group holds its qT slots for its whole
                         # span and can starve the stats pipeline)


def build_bass(t_sh=T_SH, din=DIN, o_sh=O_SH, wsc_rows=WSC_ROWS, n_w=N_W,
               n_cores=N_CORES, use_gw=False):
    """Build the per-core SPMD Bass graph. use_gw=True adds the x*norm_weight
    multiply (general path); the harness workload always has norm_weight==1."""
    import concourse.bass as bass
    import concourse.bacc as bacc
    import concourse.mybir as mybir
    from concourse import tile

    fp32 = mybir.dt.float32
    bf16 = mybir.dt.bfloat16
    Alu = mybir.AluOpType
    Act = mybir.ActivationFunctionType

    t_tiles = t_sh // P          # token tiles
    k_tiles = din // P           # contraction tiles
    oc_sz = 512 if o_sh >= 512 else o_sh
    oc_chunks = o_sh // oc_sz    # PSUM output chunks per token tile
    wsc_tiles = wsc_rows // P
    n_early = min(N_EARLY, t_tiles)
    n_group = min(N_GROUP, n_early)

    nc = bacc.Bacc("TRN2", target_bir_lowering=False, debug=False,
                   num_devices=n_cores)

    x_d = nc.dram_tensor("x", [t_sh, din], fp32, kind="ExternalInput")
    wt_d = nc.dram_tensor("wt", [din, o_sh], fp32, kind="ExternalInput")
    wsc_d = nc.dram_tensor("wsc", [wsc_rows, din], fp32, kind="ExternalInput")
    if use_gw:
        gw_d = nc.dram_tensor("gw", [P, din], fp32, kind="ExternalInput")
    out_d = nc.dram_tensor("out", [t_sh, o_sh], fp32, kind="ExternalOutput")

    # collective bounce buffers (internal DRAM)
    pin_d = nc.dram_tensor("cc_in", [P, 1], fp32)
    pout_d = nc.dram_tensor("cc_out", [P, 1], fp32)
    warm_in_d = nc.dram_tensor("cc_warm_in", [P, 1], fp32)
    warm_out_d = nc.dram_tensor("cc_warm_out", [P, 1], fp32)

    with tile.TileContext(nc) as tc:
        with (
            tc.tile_pool(name="persist", bufs=1) as persist,
            tc.tile_pool(name="xin", bufs=4) as xin_pool,
            tc.tile_pool(name="ybuf", bufs=2 if use_gw else 1) as y_pool,
            tc.tile_pool(name="wscin", bufs=2) as wsc_pool,
            tc.tile_pool(name="t1buf", bufs=2) as t1_pool,
            tc.tile_pool(name="qbuf", bufs=3) as q_pool,
            tc.tile_pool(name="qtbuf", bufs=6) as qt_pool,
            tc.tile_pool(name="obuf", bufs=3) as out_pool,
            tc.tile_pool(name="wtq", bufs=5) as wtq_pool,
            tc.tile_pool(name="small", bufs=4) as small,
            tc.tile_pool(name="psum", bufs=8, space="PSUM") as psum_pool,
        ):
            # Warm-up collective issued first (no data deps): the ncfw path
            # pays a ~50-70us arming delay plus inter-core launch skew on the
            # first collective of a NEFF. Absorb both on dummy buffers while
            # pass A's DMAs run, so the real AllReduce below pays only the
            # ~15us op cost at a predictable time.
            zsb = small.tile([P, 1], fp32, name="zsb")
            nc.gpsimd.memset(zsb[:], 0.0)
            nc.gpsimd.dma_start(warm_in_d[:], zsb[:])
            nc.gpsimd.collective_compute(
                "AllReduce", Alu.add,
                replica_groups=[list(range(n_cores))],
                ins=[warm_in_d[:]], outs=[warm_out_d[:]],
            )

            # ---------------- persistent tiles
            if use_gw:
                gw_sb = persist.tile([P, din], fp32)
                nc.scalar.dma_start(gw_sb[:], gw_d[:])
            ones_sb = persist.tile([P, P], fp32)
            # per-k quantized transposed weight blocks [d_lo, o], offset +192
            wq = [persist.tile([P, o_sh], bf16, name=f"wq{k}") for k in range(k_tiles)]
            # per-token stats, one column per token tile
            sumsq_t = persist.tile([P, t_tiles], fp32)
            amax_t = persist.tile([P, t_tiles], fp32)
            m_t = persist.tile([P, t_tiles], fp32)
            alpha_t = persist.tile([P, t_tiles], fp32)
            bias_t = persist.tile([P, t_tiles], fp32)   # -192*alpha*rowsum(q)
            wacc = persist.tile([P, wsc_tiles], fp32)

            # ---------------- pass A: global sum |w| -> early AllReduce
            # high_priority: the scheduler must place this chain at the head
            # of every engine queue, or the real AllReduce trigger ends up
            # behind x-tile work that waits on late DMAs (observed +60us).
            # The trigger-critical DMAs live on DVE (pin) and GpSimd
            # (warmup feed, readback) where the preceding queue content is
            # guaranteed early.
            with tc.high_priority():
                for j in range(wsc_tiles):
                    wtile = wsc_pool.tile([P, din], fp32, tag="wsc")
                    nc.scalar.dma_start(wtile[:], wsc_d[j * P:(j + 1) * P, :])
                    # in-place |.|: keeps the 2-slot pool truly double-buffered
                    nc.scalar.activation(wtile[:], wtile[:], Act.Abs,
                                         accum_out=wacc[:, j:j + 1])
                # final [P,8]->[P,1] on ACT too: DVE stays out of the AR path
                wpart = small.tile([P, 1], fp32)
                waccscr = small.tile([P, wsc_tiles], fp32, name="waccscr")
                nc.scalar.activation(waccscr[:], wacc[:], Act.Identity,
                                     accum_out=wpart[:])
                nc.gpsimd.dma_start(pin_d[:], wpart[:])
                nc.gpsimd.collective_compute(
                    "AllReduce", Alu.add,
                    replica_groups=[list(range(n_cores))],
                    ins=[pin_d[:]], outs=[pout_d[:]],
                )
                wsum_all = small.tile([P, 1], fp32)
                nc.gpsimd.dma_start(wsum_all[:], pout_d[:])
                nc.gpsimd.memset(ones_sb[:], 1.0)
                # cross-partition sum + broadcast via ones matmul
                psum_s = psum_pool.tile([P, oc_sz], fp32, tag="ps", name="psum_s")
                nc.tensor.matmul(psum_s[:, 0:1], ones_sb[:], wsum_all[:],
                                 start=True, stop=True)
            # AR-dependent scalar chain on ACT/Pool only: nothing here may
            # occupy the DVE queue, or the in-order queue would block the
            # early x-tile stats behind the AllReduce.
            ssum = small.tile([P, 1], fp32)
            nc.scalar.activation(ssum[:], psum_s[:, 0:1], Act.Identity)
            ws = small.tile([P, 1], fp32)   # w_scale per partition (all equal)
            nc.gpsimd.tensor_scalar(out=ws[:], in0=ssum[:], scalar1=1.0 / n_w,
                                    scalar2=EPS_SCALE, op0=Alu.mult, op1=Alu.add)
            neg_ws = small.tile([P, 1], fp32)
            nc.gpsimd.tensor_scalar(out=neg_ws[:], in0=ws[:], scalar1=-1.0,
                                    scalar2=None, op0=Alu.mult)
            woff_sb = persist.tile([P, 1], fp32, name="woff")
            nc.gpsimd.memset(woff_sb[:], W_OFF)

            # ---------------- per-tile stat/quant chain (no matmuls)
            def stats(i):
                xt = xin_pool.tile([P, din], fp32, tag="xin")
                nc.scalar.dma_start(xt[:], x_d[i * P:(i + 1) * P, :])
                if use_gw:
                    yt = y_pool.tile([P, din], fp32, tag="y")
                    nc.vector.tensor_tensor(out=yt[:], in0=xt[:], in1=gw_sb[:],
                                            op=Alu.mult)
                else:
                    yt = xt
                # ---- qT critical path: pure DVE (no cross-engine hops).
                # m = 1/(amax/QB): dropping the eps*rms term shifts the quant
                # grid by 3.5e-7 relative — rounding flips are negligible; the
                # exact per-token scale (alpha, incl. eps terms) is still used
                # for dequant below, off the critical path.
                nc.vector.tensor_reduce(out=amax_t[:, i:i + 1], in_=yt[:],
                                        op=Alu.max, axis=mybir.AxisListType.X,
                                        apply_absolute_value=True)
                d1 = small.tile([P, 1], fp32, tag="d1")
                nc.vector.tensor_scalar(out=d1[:], in0=amax_t[:, i:i + 1],
                                        scalar1=1.0 / QB, scalar2=None,
                                        op0=Alu.mult)
                nc.vector.reciprocal(m_t[:, i:i + 1], d1[:])
                # quantize x (fp32 magic round): q = round(yt * m)
                t1 = t1_pool.tile([P, din], fp32, tag="t1")
                nc.vector.tensor_scalar(out=t1[:], in0=yt[:],
                                        scalar1=m_t[:, i:i + 1],
                                        scalar2=C_MAGIC,
                                        op0=Alu.mult, op1=Alu.add)
                # qt8 pass also accumulates rowsum(q) for the +192 correction
                qt8 = q_pool.tile([P, din], bf16, tag="q")
                rs = small.tile([P, 1], fp32, tag="rs")
                nc.vector.tensor_scalar(out=qt8[:], in0=t1[:], scalar1=C_MAGIC,
                                        scalar2=0.0, op0=Alu.subtract,
                                        op1=Alu.add, accum_out=rs[:])
                # one xbar transpose for the whole tile: out[d_lo, k, t] =
                # qt8[t, 128k + d_lo]  (verified blocked layout on HW)
                qT = qt_pool.tile([P, k_tiles, P], bf16, tag="qT")
                nc.sync.dma_start(out=qT[:], in_=qt8[:], transpose=True)

                # ---- dequant scale side (deadline: this tile's PSUM copies)
                scr = t1_pool.tile([P, din], fp32, tag="t1")
                nc.scalar.activation(scr[:], xt[:], Act.Square,
                                     accum_out=sumsq_t[:, i:i + 1])
                mse = small.tile([P, 1], fp32, tag="mse")
                nc.vector.tensor_scalar(out=mse[:], in0=sumsq_t[:, i:i + 1],
                                        scalar1=1.0 / din, scalar2=EPS_NORM,
                                        op0=Alu.mult, op1=Alu.add)
                sq = small.tile([P, 1], fp32, tag="sq")
                nc.scalar.activation(sq[:], mse[:], Act.Sqrt)
                rsq = small.tile([P, 1], fp32, tag="rsq")
                nc.vector.reciprocal(rsq[:], sq[:])
                xs0 = small.tile([P, 1], fp32, tag="xs0")
                nc.vector.tensor_tensor(out=xs0[:], in0=d1[:], in1=rsq[:],
                                        op=Alu.mult)
                # alpha = (xs0 + eps) * w_scale
                nc.vector.tensor_scalar(out=alpha_t[:, i:i + 1], in0=xs0[:],
                                        scalar1=EPS_SCALE, scalar2=ws[:],
                                        op0=Alu.add, op1=Alu.mult)
                nc.vector.tensor_scalar(out=bias_t[:, i:i + 1], in0=rs[:],
                                        scalar1=alpha_t[:, i:i + 1],
                                        scalar2=-W_OFF,
                                        op0=Alu.mult, op1=Alu.mult)
                return qT

            qts = {}
            for i in range(n_early):
                qts[i] = stats(i)

            # inv_ws on DVE, emitted only after the early stats so the
            # in-order DVE queue works through them during the AR window
            inv_ws = small.tile([P, 1], fp32)
            nc.vector.reciprocal(inv_ws[:], ws[:])

            # ---------------- pass C: quantize wT -> {191,192,193} bf16
            # 2 ALU ops per [128, oc_sz] chunk, spread across DVE/Pool (clip)
            # and ACT/DVE (scale+offset, RNE on the bf16 store)
            for oc in range(oc_chunks):
                osl = slice(oc * oc_sz, (oc + 1) * oc_sz)
                for k in range(k_tiles):
                    wtile = wtq_pool.tile([P, oc_sz], fp32, tag="wtq")
                    nc.scalar.dma_start(wtile[:], wt_d[k * P:(k + 1) * P, osl])
                    ctile = wtq_pool.tile([P, oc_sz], fp32, tag="wclip", bufs=4)
                    eng = nc.vector if (k % 2 == 0) else nc.gpsimd
                    eng.tensor_scalar(out=ctile[:], in0=wtile[:],
                                      scalar1=ws[:], scalar2=neg_ws[:],
                                      op0=Alu.min, op1=Alu.max)
                    if k % 8 < 5:
                        nc.scalar.activation(wq[k][:, osl], ctile[:],
                                             Act.Identity, scale=inv_ws[:],
                                             bias=woff_sb[:])
                    else:
                        nc.vector.tensor_scalar(out=wq[k][:, osl], in0=ctile[:],
                                                scalar1=inv_ws[:], scalar2=W_OFF,
                                                op0=Alu.mult, op1=Alu.add)

            # ---------------- matmul + output block for one token tile
            def mm_oc(i, qT, oc):
                osl = slice(oc * oc_sz, (oc + 1) * oc_sz)
                pt = psum_pool.tile([P, oc_sz], fp32, tag="ps")
                for k in range(k_tiles):
                    nc.tensor.matmul(pt[:], qT[:, k, :], wq[k][:, osl],
                                     start=(k == 0), stop=(k == k_tiles - 1))
                osb = out_pool.tile([P, oc_sz], fp32, tag="o")
                nc.scalar.activation(osb[:], pt[:], Act.Identity,
                                     scale=alpha_t[:, i:i + 1],
                                     bias=bias_t[:, i:i + 1])
                nc.scalar.dma_start(out_d[i * P:(i + 1) * P, osl], osb[:])

            def mm(i, qT):
                for oc in range(oc_chunks):
                    mm_oc(i, qT, oc)

            # ---------------- early matmuls oc-major across the first group:
            # group tiles' oc-chunk deadlines then trail the quant pipeline
            # instead of demanding all 4 chunks 14us after the AllReduce.
            for oc in range(oc_chunks):
                for g in range(n_group):
                    mm_oc(g, qts[g], oc)
            for g in range(n_group):
                qts.pop(g)

            # ---------------- steady state: interleave remaining stats with mms
            # tile_wait_until: sim-only dispatch floor that forces the
            # scheduler to order these AFTER the quant pass in every engine
            # queue (its collective-latency estimate otherwise interleaves
            # late-arriving x-tile stats ahead of quant, stalling the PE).
            for i in range(n_group, t_tiles):
                j = i + n_early - n_group
                if j < t_tiles and j >= n_early:
                    with tc.tile_wait_until(0.5 + 0.013 * (j - n_early)):
                        qts[j] = stats(j)
                mm(i, qts.pop(i))

    nc.compile()
    return nc


# ---------------------------------------------------------------- host wrapper

_CACHED = {}


def _get_nc(use_gw):
    key = ("nc", use_gw)
    if key not in _CACHED:
        _CACHED[key] = build_bass(use_gw=use_gw)
    return _CACHED[key]


def kernel(x: np.ndarray, weight: np.ndarray, norm_weight: np.ndarray) -> np.ndarray:
    from concourse.bass_utils import run_bass_kernel_spmd

    assert x.shape == (B, S, DIN) and weight.shape == (DOUT, DIN)
    use_gw = not np.allclose(np.asarray(norm_weight, dtype=np.float32), 1.0)
    x_flat = np.ascontiguousarray(x.reshape(TOKENS, DIN), dtype=np.float32)
    w = np.ascontiguousarray(weight, dtype=np.float32)
    wt_full = np.ascontiguousarray(w.T)  # [DIN, DOUT]

    in_maps = []
    for c in range(N_CORES):
        tg, og = divmod(c, OG)
        m = {
            "x": np.ascontiguousarray(x_flat[tg * T_SH:(tg + 1) * T_SH]),
            "wt": np.ascontiguousarray(wt_full[:, og * O_SH:(og + 1) * O_SH]),
            "wsc": np.ascontiguousarray(w[c * WSC_ROWS:(c + 1) * WSC_ROWS]),
        }
        if use_gw:
            m["gw"] = np.ascontiguousarray(
                np.broadcast_to(norm_weight.astype(np.float32), (P, DIN)))
        in_maps.append(m)

    nc = _get_nc(use_gw)
    res = run_bass_kernel_spmd(nc, in_maps, core_ids=list(range(N_CORES)))
    _CACHED["last_results"] = res

    out = np.empty((TOKENS, DOUT), dtype=np.float32)
    for c in range(N_CORES):
        tg, og = divmod(c, OG)
        out[tg * T_SH:(tg + 1) * T_SH, og * O_SH:(og + 1) * O_SH] = \
            res.results[c]["out"]
    return out.reshape(B, S, DOUT)


# revision 42
# speedup vs baseline: 1.1194x; 1.1194x over previous
"""BitLinear (RMSNorm + int8 act quant + ternary weight quant + GEMM) on 8 TRN2 cores.

Sharding: 2 token-groups x 4 dout-groups. Each core:
  - x shard [4096, 2048] (token-parallel)
  - wT shard [2048, 2048] = weight[og*2048:(og+1)*2048, :].T  (host pre-transposed layout)
  - wsc shard [1024, 2048] = weight[c*1024:(c+1)*1024, :]     (for global mean|w| AllReduce)

Device pipeline per core (v6):
  A: warmup AllReduce issued immediately (absorbs the ~50-70us ncfw arming +
     launch skew); wsc streamed + Abs-accum; real AllReduce ~15us right after.
  B: x-tile stats for the first tiles run through the arming window; the
     per-token scalar chain lives on GpSimd, reciprocals on DVE, sumsq/sqrt
     on ACT.  norm_weight==1 fast path skips the x*gw multiply entirely.
  C: weight quant in 2 ALU ops per [128,512] chunk:
       c  = clip(w, -ws, +ws)                (DVE/Pool split by k parity)
       wq = bf16(c*inv_ws + 192)             (ACT/DVE split, RNE to {191,192,193})
     +192 keeps every value bf16-exact (ulp=1 in [128,256)).  psum then equals
     true_mm + 192*rowsum(x_q); fixed for free in the PSUM->SBUF copy:
       out = psum*alpha + (-192*alpha*rowsum(x_q))
     rowsum(x_q) rides the qt8 pass as a DVE accumulator.
  D: first two tiles' matmuls run oc-major so early oc-chunk deadlines trail
     the quant pipeline; rest per-tile, interleaved with remaining stats.
     Output DMA per 512-col chunk.
The quantized GEMM stays exact: x_q in [-127,127], offset weights in
{191,192,193} are bf16-exact, PSUM accumulates fp32 (partials ~1e6, well
under 2^24; the 192-offset costs ~1e-3 relative rounding worst case, far
inside the 2e-2 gate).
"""

import sys

if "/opt/trn_rl_repo" not in sys.path:
    sys.path.insert(0, "/opt/trn_rl_repo")

import numpy as np

# ---------------------------------------------------------------- config

N_CORES = 8
TG, OG = 2, 4            # token groups x dout groups
B, S, DIN, DOUT = 4, 2048, 2048, 8192
TOKENS = B * S           # 8192
T_SH = TOKENS // TG      # 4096 tokens per core
O_SH = DOUT // OG        # 2048 dout per core
WSC_ROWS = DOUT // N_CORES  # 1024 rows of w per core for the scale pass

P = 128                  # partitions
EPS_NORM = 1e-6
EPS_SCALE = 1e-8
QB = 127.0
C_MAGIC = 12582912.0     # 1.5 * 2^23 : float32 RNE integer-rounding constant
W_OFF = 192.0            # bf16 magic: ints exact (ulp=1) in [128, 256)
N_W = float(DOUT * DIN)  # elements of weight for the global mean

N_EARLY = 4              # x tiles with stats emitted before weight quant
N_GROUP = 2              # leading tiles run oc-major (must be << qT pool
                         # depth: the group holds its qT slots for its whole
                         # span and can starve the stats pipeline)


def build_bass(t_sh=T_SH, din=DIN, o_sh=O_SH, wsc_rows=WSC_ROWS, n_w=N_W,
               n_cores=N_CORES, use_gw=False):
    """Build the per-core SPMD Bass graph. use_gw=True adds the x*norm_weight
    multiply (general path); the harness workload always has norm_weight==1."""
    import concourse.bass as bass
    import concourse.bacc as bacc
    import concourse.mybir as mybir
    from concourse import tile

    fp32 = mybir.dt.float32
    bf16 = mybir.dt.bfloat16
    Alu = mybir.AluOpType
    Act = mybir.ActivationFunctionType

    t_tiles = t_sh // P          # token tiles
    k_tiles = din // P           # contraction tiles
    oc_sz = 512 if o_sh >= 512 else o_sh
    oc_chunks = o_sh // oc_sz    # PSUM output chunks per token tile
    wsc_tiles = wsc_rows // P
    n_early = min(N_EARLY, t_tiles)
    n_group = min(N_GROUP, n_early)

    nc = bacc.Bacc("TRN2", target_bir_lowering=False, debug=False,
                   num_devices=n_cores)

    x_d = nc.dram_tensor("x", [t_sh, din], fp32, kind="ExternalInput")
    wt_d = nc.dram_tensor("wt", [din, o_sh], fp32, kind="ExternalInput")
    wsc_d = nc.dram_tensor("wsc", [wsc_rows, din], fp32, kind="ExternalInput")
    if use_gw:
        gw_d = nc.dram_tensor("gw", [P, din], fp32, kind="ExternalInput")
    out_d = nc.dram_tensor("out", [t_sh, o_sh], fp32, kind="ExternalOutput")

    # collective bounce buffers (internal DRAM)
    pin_d = nc.dram_tensor("cc_in", [P, 1], fp32)
    pout_d = nc.dram_tensor("cc_out", [P, 1], fp32)
    warm_in_d = nc.dram_tensor("cc_warm_in", [P, 1], fp32)
    warm_out_d = nc.dram_tensor("cc_warm_out", [P, 1], fp32)

    with tile.TileContext(nc) as tc:
        with (
            tc.tile_pool(name="persist", bufs=1) as persist,
            tc.tile_pool(name="xin", bufs=4) as xin_pool,
            tc.tile_pool(name="ybuf", bufs=2 if use_gw else 1) as y_pool,
            tc.tile_pool(name="wscin", bufs=2) as wsc_pool,
            tc.tile_pool(name="t1buf", bufs=2) as t1_pool,
            tc.tile_pool(name="qbuf", bufs=3) as q_pool,
            tc.tile_pool(name="qtbuf", bufs=6) as qt_pool,
            tc.tile_pool(name="obuf", bufs=3) as out_pool,
            tc.tile_pool(name="wtq", bufs=5) as wtq_pool,
            tc.tile_pool(name="small", bufs=4) as small,
            tc.tile_pool(name="psum", bufs=8, space="PSUM") as psum_pool,
        ):
            # Warm-up collective issued first (no data deps): the ncfw path
            # pays a ~50-70us arming delay plus inter-core launch skew on the
            # first collective of a NEFF. Absorb both on dummy buffers while
            # pass A's DMAs run, so the real AllReduce below pays only the
            # ~15us op cost at a predictable time.
            zsb = small.tile([P, 1], fp32, name="zsb")
            nc.gpsimd.memset(zsb[:], 0.0)
            nc.gpsimd.dma_start(warm_in_d[:], zsb[:])
            nc.gpsimd.collective_compute(
                "AllReduce", Alu.add,
                replica_groups=[list(range(n_cores))],
                ins=[warm_in_d[:]], outs=[warm_out_d[:]],
            )

            # ---------------- persistent tiles
            if use_gw:
                gw_sb = persist.tile([P, din], fp32)
                nc.scalar.dma_start(gw_sb[:], gw_d[:])
            ones_sb = persist.tile([P, P], fp32)
            # per-k quantized transposed weight blocks [d_lo, o], offset +192
            wq = [persist.tile([P, o_sh], bf16, name=f"wq{k}") for k in range(k_tiles)]
            # per-token stats, one column per token tile
            sumsq_t = persist.tile([P, t_tiles], fp32)
            amax_t = persist.tile([P, t_tiles], fp32)
            m_t = persist.tile([P, t_tiles], fp32)
            alpha_t = persist.tile([P, t_tiles], fp32)
            bias_t = persist.tile([P, t_tiles], fp32)   # -192*alpha*rowsum(q)
            wacc = persist.tile([P, wsc_tiles], fp32)

            # ---------------- pass A: global sum |w| -> early AllReduce
            # high_priority: the scheduler must place this chain at the head
            # of every engine queue, or the real AllReduce trigger ends up
            # behind x-tile work that waits on late DMAs (observed +60us).
            # The trigger-critical DMAs live on DVE (pin) and GpSimd
            # (warmup feed, readback) where the preceding queue content is
            # guaranteed early.
            with tc.high_priority():
                for j in range(wsc_tiles):
                    wtile = wsc_pool.tile([P, din], fp32, tag="wsc")
                    nc.scalar.dma_start(wtile[:], wsc_d[j * P:(j + 1) * P, :])
                    # in-place |.|: keeps the 2-slot pool truly double-buffered
                    nc.scalar.activation(wtile[:], wtile[:], Act.Abs,
                                         accum_out=wacc[:, j:j + 1])
                # final [P,8]->[P,1] on ACT too: DVE stays out of the AR path
                wpart = small.tile([P, 1], fp32)
                waccscr = small.tile([P, wsc_tiles], fp32, name="waccscr")
                nc.scalar.activation(waccscr[:], wacc[:], Act.Identity,
                                     accum_out=wpart[:])
                nc.gpsimd.dma_start(pin_d[:], wpart[:])
                nc.gpsimd.collective_compute(
                    "AllReduce", Alu.add,
                    replica_groups=[list(range(n_cores))],
                    ins=[pin_d[:]], outs=[pout_d[:]],
                )
                wsum_all = small.tile([P, 1], fp32)
                nc.gpsimd.dma_start(wsum_all[:], pout_d[:])
                nc.gpsimd.memset(ones_sb[:], 1.0)
                # cross-partition sum + broadcast via ones matmul
                psum_s = psum_pool.tile([P, oc_sz], fp32, tag="ps", name="psum_s")
                nc.tensor.matmul(psum_s[:, 0:1], ones_sb[:], wsum_all[:],
                                 start=True, stop=True)
            # AR-dependent scalar chain on ACT/Pool only: nothing here may
            # occupy the DVE queue, or the in-order queue would block the
            # early x-tile stats behind the AllReduce.
            ssum = small.tile([P, 1], fp32)
            nc.scalar.activation(ssum[:], psum_s[:, 0:1], Act.Identity)
            ws = small.tile([P, 1], fp32)   # w_scale per partition (all equal)
            nc.gpsimd.tensor_scalar(out=ws[:], in0=ssum[:], scalar1=1.0 / n_w,
                                    scalar2=EPS_SCALE, op0=Alu.mult, op1=Alu.add)
            neg_ws = small.tile([P, 1], fp32)
            nc.gpsimd.tensor_scalar(out=neg_ws[:], in0=ws[:], scalar1=-1.0,
                                    scalar2=None, op0=Alu.mult)
            woff_sb = persist.tile([P, 1], fp32, name="woff")
            nc.gpsimd.memset(woff_sb[:], W_OFF)

            # ---------------- per-tile stat/quant chain (no matmuls)
            def stats(i):
                xt = xin_pool.tile([P, din], fp32, tag="xin")
                nc.scalar.dma_start(xt[:], x_d[i * P:(i + 1) * P, :])
                if use_gw:
                    yt = y_pool.tile([P, din], fp32, tag="y")
                    nc.vector.tensor_tensor(out=yt[:], in0=xt[:], in1=gw_sb[:],
                                            op=Alu.mult)
                else:
                    yt = xt
                # ---- qT critical path: pure DVE (no cross-engine hops).
                # m = 1/(amax/QB): dropping the eps*rms term shifts the quant
                # grid by 3.5e-7 relative — rounding flips are negligible; the
                # exact per-token scale (alpha, incl. eps terms) is still used
                # for dequant below, off the critical path.
                nc.vector.tensor_reduce(out=amax_t[:, i:i + 1], in_=yt[:],
                                        op=Alu.max, axis=mybir.AxisListType.X,
                                        apply_absolute_value=True)
                d1 = small.tile([P, 1], fp32, tag="d1")
                nc.vector.tensor_scalar(out=d1[:], in0=amax_t[:, i:i + 1],
                                        scalar1=1.0 / QB, scalar2=None,
                                        op0=Alu.mult)
                nc.vector.reciprocal(m_t[:, i:i + 1], d1[:])
                # quantize x (fp32 magic round): q = round(yt * m)
                t1 = t1_pool.tile([P, din], fp32, tag="t1")
                nc.vector.tensor_scalar(out=t1[:], in0=yt[:],
                                        scalar1=m_t[:, i:i + 1],
                                        scalar2=C_MAGIC,
                                        op0=Alu.mult, op1=Alu.add)
                # qt8 pass also accumulates rowsum(q) for the +192 correction
                qt8 = q_pool.tile([P, din], bf16, tag="q")
                rs = small.tile([P, 1], fp32, tag="rs")
                nc.vector.tensor_scalar(out=qt8[:], in0=t1[:], scalar1=C_MAGIC,
                                        scalar2=0.0, op0=Alu.subtract,
                                        op1=Alu.add, accum_out=rs[:])
                # one xbar transpose for the whole tile: out[d_lo, k, t] =
                # qt8[t, 128k + d_lo]  (verified blocked layout on HW)
                qT = qt_pool.tile([P, k_tiles, P], bf16, tag="qT")
                nc.sync.dma_start(out=qT[:], in_=qt8[:], transpose=True)

                # ---- dequant scale side (deadline: this tile's PSUM copies)
                scr = t1_pool.tile([P, din], fp32, tag="t1")
                nc.scalar.activation(scr[:], xt[:], Act.Square,
                                     accum_out=sumsq_t[:, i:i + 1])
                mse = small.tile([P, 1], fp32, tag="mse")
                nc.vector.tensor_scalar(out=mse[:], in0=sumsq_t[:, i:i + 1],
                                        scalar1=1.0 / din, scalar2=EPS_NORM,
                                        op0=Alu.mult, op1=Alu.add)
                sq = small.tile([P, 1], fp32, tag="sq")
                nc.scalar.activation(sq[:], mse[:], Act.Sqrt)
                rsq = small.tile([P, 1], fp32, tag="rsq")
                nc.vector.reciprocal(rsq[:], sq[:])
                xs0 = small.tile([P, 1], fp32, tag="xs0")
                nc.vector.tensor_tensor(out=xs0[:], in0=d1[:], in1=rsq[:],
                                        op=Alu.mult)
                # alpha = (xs0 + eps) * w_scale
                nc.vector.tensor_scalar(out=alpha_t[:, i:i + 1], in0=xs0[:],
                                        scalar1=EPS_SCALE, scalar2=ws[:],
                                        op0=Alu.add, op1=Alu.mult)
                nc.vector.tensor_scalar(out=bias_t[:, i:i + 1], in0=rs[:],
                                        scalar1=alpha_t[:, i:i + 1],
                                        scalar2=-W_OFF,
                                        op0=Alu.mult, op1=Alu.mult)
                return qT

            qts = {}
            for i in range(n_early):
                qts[i] = stats(i)

            # inv_ws on DVE (reciprocal is DVE-only), emitted after the early
            # stats; the only DVE op the weight path still needs
            inv_ws = small.tile([P, 1], fp32)
            nc.vector.reciprocal(inv_ws[:], ws[:])

            # ---------------- pass C: quantize wT -> {191,192,193} bf16
            # 2 ALU ops per [128, oc_sz] chunk, spread across DVE/Pool (clip)
            # and ACT/DVE (scale+offset, RNE on the bf16 store)
            for oc in range(oc_chunks):
                osl = slice(oc * oc_sz, (oc + 1) * oc_sz)
                for k in range(k_tiles):
                    # Pool clip + ACT scale only: the DVE queue stays free of
                    # AR-gated work, so the scheduler cannot order late x-tile
                    # stats ahead of quant on it (cost 50us when it did).
                    wtile = wtq_pool.tile([P, oc_sz], fp32, tag="wtq")
                    nc.scalar.dma_start(wtile[:], wt_d[k * P:(k + 1) * P, osl])
                    ctile = wtq_pool.tile([P, oc_sz], fp32, tag="wclip", bufs=4)
                    nc.gpsimd.tensor_scalar(out=ctile[:], in0=wtile[:],
                                            scalar1=ws[:], scalar2=neg_ws[:],
                                            op0=Alu.min, op1=Alu.max)
                    nc.scalar.activation(wq[k][:, osl], ctile[:],
                                         Act.Identity, scale=inv_ws[:],
                                         bias=woff_sb[:])

            # ---------------- matmul + output block for one token tile
            def mm_oc(i, qT, oc):
                osl = slice(oc * oc_sz, (oc + 1) * oc_sz)
                pt = psum_pool.tile([P, oc_sz], fp32, tag="ps")
                for k in range(k_tiles):
                    nc.tensor.matmul(pt[:], qT[:, k, :], wq[k][:, osl],
                                     start=(k == 0), stop=(k == k_tiles - 1))
                osb = out_pool.tile([P, oc_sz], fp32, tag="o")
                nc.scalar.activation(osb[:], pt[:], Act.Identity,
                                     scale=alpha_t[:, i:i + 1],
                                     bias=bias_t[:, i:i + 1])
                nc.scalar.dma_start(out_d[i * P:(i + 1) * P, osl], osb[:])

            def mm(i, qT):
                for oc in range(oc_chunks):
                    mm_oc(i, qT, oc)

            # ---------------- early matmuls oc-major across the first group:
            # group tiles' oc-chunk deadlines then trail the quant pipeline
            # instead of demanding all 4 chunks 14us after the AllReduce.
            for oc in range(oc_chunks):
                for g in range(n_group):
                    mm_oc(g, qts[g], oc)
            for g in range(n_group):
                qts.pop(g)

            # ---------------- steady state: interleave remaining stats with mms
            for i in range(n_group, t_tiles):
                j = i + n_early - n_group
                if j < t_tiles and j >= n_early:
                    qts[j] = stats(j)
                mm(i, qts.pop(i))

    nc.compile()
    return nc


# ---------------------------------------------------------------- host wrapper

_CACHED = {}


def _get_nc(use_gw):
    key = ("nc", use_gw)
    if key not in _CACHED:
        _CACHED[key] = build_bass(use_gw=use_gw)
    return _CACHED[key]


def kernel(x: np.ndarray, weight: np.ndarray, norm_weight: np.ndarray) -> np.ndarray:
    from concourse.bass_utils import run_bass_kernel_spmd

    assert x.shape == (B, S, DIN) and weight.shape == (DOUT, DIN)
    use_gw = not np.allclose(np.asarray(norm_weight, dtype=np.float32), 1.0)
    x_flat = np.ascontiguousarray(x.reshape(TOKENS, DIN), dtype=np.float32)
    w = np.ascontiguousarray(weight, dtype=np.float32)
    wt_full = np.ascontiguousarray(w.T)  # [DIN, DOUT]

    in_maps = []
    for c in range(N_CORES):
        tg, og = divmod(c, OG)
        m = {
            "x": np.ascontiguousarray(x_flat[tg * T_SH:(tg + 1) * T_SH]),
            "wt": np.ascontiguousarray(wt_full[:, og * O_SH:(og + 1) * O_SH]),
            "wsc": np.ascontiguousarray(w[c * WSC_ROWS:(c + 1) * WSC_ROWS]),
        }
        if use_gw:
            m["gw"] = np.ascontiguousarray(
                np.broadcast_to(norm_weight.astype(np.float32), (P, DIN)))
        in_maps.append(m)

    nc = _get_nc(use_gw)
    res = run_bass_kernel_spmd(nc, in_maps, core_ids=list(range(N_CORES)))
    _CACHED["last_results"] = res

    out = np.empty((TOKENS, DOUT), dtype=np.float32)
    for c in range(N_CORES):
        tg, og = divmod(c, OG)
        out[tg * T_SH:(tg + 1) * T_SH, og * O_SH:(og + 1) * O_SH] = \
            res.results[c]["out"]
    return out.reshape(B, S, DOUT)
